# revision 37
# baseline (speedup 1.0000x reference)
"""CapsuleLayer Bass/Tile kernel for TRN2 (one NeuronCore; replicated SPMD x8).

Host-side prep transposes+casts x to fp16 in a per-b-tile layout so the
device does no transposes at all:
  xin[t*128 + p, j*128 + b] = x[t*128 + b, j*128 + p]   (fp16)
Per 128-sample b-tile t, the 16 chunks j are matmul lhsT operands
directly: out[b, c] += sum_p xin_t[p, j*128+b] * kpad[p, j*PADN+c].

kpad (fp16) holds the [2048, 160] kernel as [128, 16*PADN] with col
160:176 of each chunk = 0.1 * sum over capsule blocks, so the first
routing iteration's weighted sum falls out of the matmul.

Routing runs on [128, G*160] group-wide tiles (DVE/ACT/GPSIMD). sqrt is
computed as exp(0.5*ln(x)) so every ACT func (Copy/Ln/Exp) lives in one
activation table set (no LoadActFuncSet thrash).
"""

from dataclasses import dataclass

import numpy as np

import concourse.bacc as bacc
import concourse.tile as tile
from concourse import mybir

NCAP = 10
DCAP = 16
EPS = 1e-7
D = 2048
NCOL = NCAP * DCAP  # 160
NCHUNK = 16  # 2048 / 128


@dataclass
class Cfg:
    n_btiles: int = 16          # 128-sample tiles per core
    group: int = 4              # b-tiles per routing group
    pad_n: int = 176            # kpad columns per chunk (160 hat + 16 s1)
    n_cores: int = 8
    reps: int = 1               # repeat whole pipeline (for slope timing)
    dma_btiles: int = 1         # b-tiles per input DMA
    loop_reps: int = 0          # >0: wrap body in a hardware For_i loop
    group_sizes: str = ""       # e.g. "8,5,3"; overrides group when set
    big_pat: str = "ddpdddpd"   # engine per big routing op (p=Pool, d=DVE)
    pump: int = 8               # routing steps pumped per b-tile emitted
    x_bufs: int = 6
    phat_bufs: int = 8
    hatw_bufs: int = 3
    rt_bufs: int = 3
    sm_bufs: int = 3
    outs_bufs: int = 3
    copy_eng: str = "a"         # PSUM->SBUF copy engine: a=ACT p=Pool, or
                                # a multi-char pattern cycled per b-tile
    small_eng: str = "d"        # engine for squash small muls/stt (d/p)
    red_split: int = 0          # 1: halve big reduces with a Pool add stage
    sub3: int = 0               # 1: subtract max before softmax3 (f16 iter3)
    load_eng: str = "s"         # xin load queue per b-tile: s=SP a=ACT,
                                # multi-char pattern cycled
    store_eng: str = "a"        # yout store queue: a=ACT hwdge, p=Pool swdge
    yraw: int = 0               # 1: store yout as [128, NB*DCAP] (contiguous
                                # per-partition runs; host unshuffles)

    @property
    def bc(self):
        return self.n_btiles * 128


def prep_x(x_core: np.ndarray, n_btiles: int) -> np.ndarray:
    """[Bc, 2048] f32 -> [Bc, 2048] fp16 laid out [t*128+p, j*128+b]."""
    bc = x_core.shape[0]
    assert x_core.shape == (bc, D) and bc == n_btiles * 128
    a = x_core.reshape(n_btiles, 128, NCHUNK, 128)  # [t, b, j, p]
    a = np.ascontiguousarray(a.transpose(0, 3, 2, 1))  # [t, p, j, b]
    return a.reshape(bc, D).astype(np.float16)


def make_kpad(kernel: np.ndarray, pad_n: int) -> np.ndarray:
    """[2048, 160] f32 -> [128, 16*pad_n] fp16, kpad[p, j*pad_n+c] =
    k[j*128+p, c]; col 160:176 = 0.1 * sum over capsules; rest zero."""
    d, ncol = kernel.shape
    assert (d, ncol) == (D, NCOL)
    kp = np.zeros((NCHUNK, 128, pad_n), dtype=np.float32)
    kj = kernel.reshape(NCHUNK, 128, NCOL)
    kp[:, :, :NCOL] = kj
    kp[:, :, NCOL:NCOL + DCAP] = 0.1 * kj.reshape(NCHUNK, 128, NCAP, DCAP).sum(axis=2)
    out = kp.transpose(1, 0, 2).reshape(128, NCHUNK * pad_n)
    return np.ascontiguousarray(out).astype(np.float16)


def build(cfg: Cfg):
    nc = bacc.Bacc("TRN2", target_bir_lowering=False, debug=False,
                   num_devices=cfg.n_cores)
    f16 = mybir.dt.float16
    f32 = mybir.dt.float32

    NB = cfg.n_btiles
    PADN = cfg.pad_n
    if cfg.group_sizes:
        sizes = [int(s) for s in cfg.group_sizes.split(",")]
    else:
        assert NB % cfg.group == 0
        sizes = [cfg.group] * (NB // cfg.group)
    assert sum(sizes) == NB

    eps_t = nc.alloc_sbuf_tensor("const-eps", [128, 1], f32)
    nc.gpsimd.memset(eps_t.ap(), EPS)
    nc.const_aps.aps[(f32, EPS)] = eps_t.ap()
    nc.all_engine_barrier()

    xin = nc.dram_tensor("xin", [cfg.bc, D], f16, kind="ExternalInput")
    kpad = nc.dram_tensor("kpad", [128, NCHUNK * PADN], f16,
                          kind="ExternalInput")
    # f16 output (host upcasts): halves store traffic; output absmax ~0.8
    # so f16 quantization (~5e-4 rel) is negligible vs the 2e-2 budget
    if cfg.yraw:
        # [p, t*16+d] = out[t*128+p, d]: 256B-contiguous per-partition runs
        # per group store instead of 32B rows (host unshuffles)
        yout = nc.dram_tensor("yout", [128, cfg.n_btiles * DCAP], f16,
                              kind="ExternalOutput")
    else:
        yout = nc.dram_tensor("yout", [cfg.bc, DCAP], f16,
                              kind="ExternalOutput")

    with tile.TileContext(nc) as tc:
        with (
            tc.tile_pool(name="const", bufs=1) as constp,
            tc.tile_pool(name="xt", bufs=cfg.x_bufs) as xtp,
            tc.tile_pool(name="phat", bufs=cfg.phat_bufs, space="PSUM") as phatp,
            tc.tile_pool(name="hatw", bufs=cfg.hatw_bufs) as hatwp,
            tc.tile_pool(name="rt", bufs=cfg.rt_bufs) as rtp,
            tc.tile_pool(name="sm", bufs=cfg.sm_bufs) as smp,
            tc.tile_pool(name="outs", bufs=cfg.outs_bufs) as outsp,
        ):
            # kpad is loop-invariant: load once, outside any timing loop
            kp_t = constp.tile([128, NCHUNK * PADN], f16, tag="kpad")
            nc.sync.dma_start(kp_t[:], kpad[:, :])

            xv = xin[:, :].rearrange("(t p) d -> t p d", p=128)

            DB = cfg.dma_btiles
            x_slices = {}  # i -> (tile, col offset)

            def load_chunk(i0):
                xt = xtp.tile([128, DB * D], f16, tag="xt")
                le = cfg.load_eng[(i0 // DB) % len(cfg.load_eng)]
                eng = nc.scalar if le == "a" else nc.sync
                if DB == 1:
                    eng.dma_start(xt[:], xv[i0])
                else:
                    eng.dma_start(
                        xt[:].rearrange("p (t d) -> p t d", t=DB),
                        xin[:, :].rearrange("(c t p) d -> c p t d",
                                            t=DB, p=128)[i0 // DB],
                    )
                for t in range(DB):
                    x_slices[i0 + t] = (xt, t * D)

            S1C = PADN  # 176 = 11 * 16 per-tile block in hatw

            def big_engines():
                """Engine pattern for the 8 big [128, G*160] ops per group:
                order of use: mul2,red2, mulw2,redw2, mul3,red3, mulw3,redw3."""
                pat = []
                for ch in cfg.big_pat:
                    pat.append(nc.gpsimd if ch == "p" else nc.vector)
                return pat

            def routing_gen(i0, G, gi, hatw):
                """Generator emitting routing ops for one group; yields
                between ops so the driver can interleave groups (wavefront
                order keeps every engine queue head runnable)."""
                if cfg.yraw:
                    yv = yout[:, i0 * DCAP:(i0 + G) * DCAP].rearrange(
                        "p (g d) -> p g d", g=G)
                else:
                    yv = yout[i0 * 128:(i0 + G) * 128, :].rearrange(
                        "(g p) d -> p g d", p=128)
                W = hatw[:].rearrange("p (g n d) -> p g n d", n=NCAP + 1,
                                      d=DCAP)
                H = W[:, :, 0:NCAP, :]
                S1 = W[:, :, NCAP, :]
                eng = big_engines()

                def sqrt_eps(n2, tag):
                    lg = smp.tile([128, G], f32, tag=f"lg{tag}")
                    nc.scalar.activation(lg[:], n2,
                                         mybir.ActivationFunctionType.Ln,
                                         bias=EPS)
                    sr = smp.tile([128, G], f32, tag=f"sr{tag}")
                    nc.scalar.activation(sr[:], lg[:],
                                         mybir.ActivationFunctionType.Exp,
                                         scale=0.5)
                    return sr

                sm_eng = nc.gpsimd if cfg.small_eng == "p" else nc.vector

                def squash_steps(su, sdt, r, tag, out):
                    """out[0] = comb scale c s.t. v = c*su*(r or 1)."""
                    sq = smp.tile([128, G * DCAP], sdt, tag=f"sq{tag}")
                    nc.scalar.activation(sq[:], su,
                                         mybir.ActivationFunctionType.Square)
                    yield
                    m2 = smp.tile([128, G], f32, tag=f"m2{tag}")
                    nc.vector.tensor_reduce(
                        m2[:], sq[:].rearrange("p (g d) -> p g d", g=G),
                        axis=mybir.AxisListType.X, op=mybir.AluOpType.add)
                    yield
                    if r is not None:
                        rr = smp.tile([128, G], f32, tag=f"rr{tag}")
                        sm_eng.tensor_mul(rr[:], r, r)
                        n2 = smp.tile([128, G], f32, tag=f"n2{tag}")
                        sm_eng.tensor_mul(n2[:], m2[:], rr[:])
                        yield
                    else:
                        n2 = m2
                    sr = sqrt_eps(n2[:], tag)
                    yield
                    den = smp.tile([128, G], f32, tag=f"den{tag}")
                    nc.vector.scalar_tensor_tensor(
                        den[:], n2[:], 1.0, sr[:],
                        op0=mybir.AluOpType.add, op1=mybir.AluOpType.mult)
                    rec = smp.tile([128, G], f32, tag=f"rec{tag}")
                    nc.vector.reciprocal(rec[:], den[:])
                    yield
                    sc = smp.tile([128, G], f32, tag=f"sc{tag}")
                    sm_eng.tensor_mul(sc[:], n2[:], rec[:])
                    out[1] = sc
                    if r is not None:
                        comb = smp.tile([128, G], f32, tag=f"comb{tag}")
                        sm_eng.tensor_mul(comb[:], sc[:], r)
                        out[0] = comb
                    else:
                        out[0] = sc
                    yield

                def dots_steps(src_gd, e0, e1, tag, out):
                    """r[g,n] = sum_d H[g,n,d]*src[g,d] -> [128, G*NCAP] f32"""
                    tmp = rtp.tile([128, G * NCOL], f16, tag=f"dt{tag}")
                    bc = src_gd.unsqueeze(2).broadcast_to((128, G, NCAP, DCAP))
                    e0.tensor_mul(
                        tmp[:].rearrange("p (g n d) -> p g n d", g=G, n=NCAP),
                        H, bc)
                    yield
                    v = tmp[:].rearrange("p (g n d) -> p g n d", g=G, n=NCAP)
                    if cfg.red_split:
                        half = rtp.tile([128, G * NCAP * (DCAP // 2)], f16,
                                        tag=f"dh{tag}")
                        hv = half[:].rearrange("p (g n d) -> p g n d",
                                               g=G, n=NCAP)
                        with nc.allow_low_precision(reason="f16 pair sums"):
                            nc.gpsimd.tensor_add(hv, v[:, :, :, 0:DCAP // 2],
                                                 v[:, :, :, DCAP // 2:DCAP])
                        v = hv
                        yield
                    o = rtp.tile([128, G * NCAP], f32, tag=f"dr{tag}")
                    e1.tensor_reduce(
                        o[:], v, axis=mybir.AxisListType.X,
                        op=mybir.AluOpType.add)
                    out[0] = o
                    yield

                def wsum_steps(e_gn, edt, e0, e1, tag, out):
                    """su[g,d] = sum_n H[g,n,d]*e[g,n] -> [128, G*DCAP] edt"""
                    tmp = rtp.tile([128, G * NCOL], edt, tag=f"wt{tag}")
                    bc = e_gn.unsqueeze(3).broadcast_to((128, G, NCAP, DCAP))
                    e0.tensor_mul(
                        tmp[:].rearrange("p (g n d) -> p g n d", g=G, n=NCAP),
                        H, bc)
                    yield
                    v = tmp[:].rearrange("p (g n d) -> p g d n", g=G, n=NCAP)
                    if cfg.red_split:
                        half = rtp.tile([128, G * (NCAP // 2) * DCAP], edt,
                                        tag=f"wh{tag}")
                        hv = half[:].rearrange("p (g n d) -> p g d n",
                                               g=G, n=NCAP // 2)
                        with nc.allow_low_precision(reason="pair sums; rel "
                                                    "tol 2e-2"):
                            nc.gpsimd.tensor_add(
                                hv.rearrange("p g d n -> p g n d"),
                                tmp[:].rearrange("p (g n d) -> p g n d",
                                                 g=G, n=NCAP)[:, :, 0:5, :],
                                tmp[:].rearrange("p (g n d) -> p g n d",
                                                 g=G, n=NCAP)[:, :, 5:10, :])
                        v = hv
                        yield
                    o = rtp.tile([128, G * DCAP], edt, tag=f"ws{tag}")
                    with nc.allow_low_precision(reason="f16 weighted sum fits "
                                                "range; rel tol 2e-2"):
                        e1.tensor_reduce(
                            o[:], v, axis=mybir.AxisListType.X,
                            op=mybir.AluOpType.add)
                    out[0] = o
                    yield

                def softmax_steps(t_gn, edt, tag, out):
                    """e = exp(t) [128, G*NCAP]; r = 1/sum_n e [128, G]"""
                    e = rtp.tile([128, G * NCAP], edt, tag=f"e{tag}")
                    nc.scalar.activation(e[:], t_gn,
                                         mybir.ActivationFunctionType.Exp)
                    yield
                    se = smp.tile([128, G], f32, tag=f"se{tag}")
                    nc.vector.tensor_reduce(
                        se[:], e[:].rearrange("p (g n) -> p g n", g=G),
                        axis=mybir.AxisListType.X, op=mybir.AluOpType.add)
                    ri = smp.tile([128, G], f32, tag=f"ri{tag}")
                    nc.vector.reciprocal(ri[:], se[:])
                    out[0] = (e, ri)
                    yield

                gv = lambda ap: ap.rearrange("p (g d) -> p g d", g=G)
                nv = lambda ap: ap.rearrange("p (g n) -> p g n", g=G)

                # iter 1: s1 (pre-scaled mean) came from the matmul (fp16)
                c1, r2o = [None, None], [None]
                yield from squash_steps(S1, f16, None, "1", c1)
                yield from dots_steps(S1, eng[0], eng[1], "2", r2o)
                t2 = rtp.tile([128, G * NCAP], f32, tag="t2")
                nc.gpsimd.tensor_mul(
                    nv(t2[:]), nv(r2o[0][:]),
                    c1[0][:].unsqueeze(2).broadcast_to((128, G, NCAP)))
                yield

                # iter 2
                sm2 = [None]
                yield from softmax_steps(t2[:], f16, "2", sm2)
                e2, r2i = sm2[0]
                s2o = [None]
                yield from wsum_steps(nv(e2[:]), f16, eng[2], eng[3], "2", s2o)
                s2u = s2o[0]
                # dots run on the UNnormalized f16 s2u; the softmax norm r2i
                # is folded into the comb scale (c2[0] = sc*r2i) at t3.
                c2, r3o = [None, None], [None]
                yield from squash_steps(s2u[:], f32, r2i[:], "2", c2)
                yield from dots_steps(gv(s2u[:]), eng[4], eng[5], "3", r3o)
                t3 = rtp.tile([128, G * NCAP], f32, tag="t3")
                nc.gpsimd.tensor_mul(
                    nv(t3[:]), nv(r3o[0][:]),
                    c2[0][:].unsqueeze(2).broadcast_to((128, G, NCAP)))
                nc.gpsimd.tensor_add(t3[:], t3[:], t2[:])
                yield

                # iter 3: b3 logits reach ~16 so exp(b3) would overflow f16.
                # sub3: subtract the per-sample max (softmax-invariant) so
                # e3 <= 1 and the whole iteration runs in f16; else f32.
                if cfg.sub3:
                    m3 = smp.tile([128, G], f32, tag="m3")
                    nc.vector.tensor_reduce(
                        m3[:], nv(t3[:]), axis=mybir.AxisListType.X,
                        op=mybir.AluOpType.max)
                    yield
                    t3s = rtp.tile([128, G * NCAP], f16, tag="t3s")
                    with nc.allow_low_precision(reason="shifted logits are "
                                                "small"):
                        nc.gpsimd.tensor_sub(
                            nv(t3s[:]), nv(t3[:]),
                            m3[:].unsqueeze(2).broadcast_to((128, G, NCAP)))
                    yield
                    t3in, dt3 = t3s, f16
                else:
                    t3in, dt3 = t3, f32
                sm3 = [None]
                yield from softmax_steps(t3in[:], dt3, "3", sm3)
                e3, r3i = sm3[0]
                s3o = [None]
                yield from wsum_steps(nv(e3[:]), dt3, eng[6], eng[7], "3", s3o)
                s3u = s3o[0]
                c3 = [None, None]
                yield from squash_steps(s3u[:], f32, r3i[:], "3", c3)
                v3 = outsp.tile([128, G * DCAP], f16, tag="v3")
                with nc.allow_low_precision(reason="f16 output store"):
                    nc.vector.tensor_mul(
                        gv(v3[:]), gv(s3u[:]),
                        c3[0][:].unsqueeze(2).broadcast_to((128, G, DCAP)))
                # store off the SP queue so next-iteration xin loads are
                # never stuck behind routing-dependent stores
                st_eng = nc.gpsimd if cfg.store_eng == "p" else nc.scalar
                st_eng.dma_start(
                    yv,
                    v3[:].rearrange("p (g d) -> p g d", g=G))
                yield

            def run_all():
                active = []

                def pump(k):
                    for _ in range(k):
                        for gen in list(active):
                            try:
                                next(gen)
                            except StopIteration:
                                active.remove(gen)

                i0 = 0
                for gi, G in enumerate(sizes):
                    hatw = hatwp.tile([128, G * S1C], f16, tag="hatw")
                    for g in range(G):
                        i = i0 + g
                        if i % DB == 0:
                            load_chunk(i)
                        xt, off = x_slices.pop(i)
                        ph = phatp.tile([128, PADN], f32, tag="phat")
                        for j in range(NCHUNK):
                            nc.tensor.matmul(
                                ph[:],
                                xt[:, off + j * 128:off + (j + 1) * 128],
                                kp_t[:, j * PADN:(j + 1) * PADN],
                                start=(j == 0),
                                stop=(j == NCHUNK - 1),
                            )
                        ce = cfg.copy_eng[i % len(cfg.copy_eng)]
                        dst = hatw[:, g * S1C:(g + 1) * S1C]
                        if ce == "p":
                            nc.gpsimd.tensor_copy(dst, ph[:, :S1C])
                        elif ce == "v":
                            nc.vector.tensor_copy(dst, ph[:, :S1C])
                        else:
                            nc.scalar.copy(dst, ph[:, :S1C])
                        pump(cfg.pump)
                    active.append(routing_gen(i0, G, gi, hatw))
                    i0 += G
                while active:
                    pump(1)

            if cfg.loop_reps > 0:
                # hardware loop around an unrolled body: the For_i barrier
                # cost amortizes over cfg.reps unrolled pipeline repeats
                with tc.For_i(0, cfg.loop_reps, 1,
                              hint_engines=(mybir.EngineType.PE,)):
                    for _rep in range(cfg.reps):
                        run_all()
            else:
                for _rep in range(cfg.reps):
                    run_all()

    nc.compile()
    _unify_act_tables(nc)
    return nc


def _unify_act_tables(nc):
    """Replace the greedy per-func LoadActFuncSet placement with a single
    load of a set containing every activation func this kernel uses
    (Copy/Identity/Square/Exp/Ln all live in natural_log_exp_and_others).
    The greedy pass alternates exp_and_others <-> natural_log per squash,
    costing ~1.3us per reload on the ACT engine."""
    from concourse.hw_specs import get_activation_tables

    AF = mybir.ActivationFunctionType
    needed = {AF.Copy, AF.Identity, AF.Square, AF.Exp, AF.Ln}
    tables = list(get_activation_tables(nc.m.arch).items())
    combined_id = None
    for idx, (name, funcs) in enumerate(tables):
        if needed <= funcs:
            combined_id = idx
            break
    if combined_id is None:
        return  # no single set covers us; keep the pass's placement

    fn = nc.m.functions[0]
    blocks = list(fn.blocks)
    kept = None
    for blk in blocks:
        insts = list(blk.instructions)
        out = []
        for i in insts:
            if isinstance(i, mybir.InstLoadActFuncSet):
                if kept is None:
                    i.act_func_set_id = combined_id
                    kept = i
                continue  # drop (re-inserted once below)
            out.append(i)
        if len(out) != len(insts):
            blk.instructions[:] = out
    if kept is None:
        return
    # Place the single load in the entry block (before its terminator) so
    # it dominates every activation, including those inside For_i loops.
    entry = blocks[0]
    insts = list(entry.instructions)
    pos = len(insts)
    while pos > 0 and insts[pos - 1].opcode in ("UnconditionalBranch",
                                                "ConditionalBranch"):
        pos -= 1
    entry.instructions[:] = insts[:pos] + [kept] + insts[pos:]


# ---------------- numpy reference (per-core) ----------------

def ref_numpy(x: np.ndarray, kernel: np.ndarray) -> np.ndarray:
    b = x.shape[0]
    hat = (x @ kernel).reshape(b, NCAP, DCAP)
    logits = np.zeros((b, NCAP, 1), dtype=x.dtype)
    out = None
    for _ in range(3):
        ex = np.exp(logits - logits.max(axis=1, keepdims=True))
        c = ex / ex.sum(axis=1, keepdims=True)
        s = (c * hat).sum(axis=1, keepdims=True)
        s2 = np.square(s).sum(axis=-1, keepdims=True)
        out = s2 / (1.0 + s2) / np.sqrt(s2 + EPS) * s
        logits = logits + np.einsum("bnd,bd->bn", hat, out[:, 0, :])[:, :, None]
    return out[:, 0, :]


# ---------------- public entry point ----------------

_CACHE = {}

BEST = Cfg(n_btiles=16, group_sizes="8,8", big_pat="dddddddd", yraw=1)


def prep_in_maps(x: np.ndarray, kern: np.ndarray, cfg: Cfg):
    kpad = make_kpad(np.asarray(kern, dtype=np.float32), cfg.pad_n)
    return [
        {"xin": prep_x(x[i * cfg.bc:(i + 1) * cfg.bc], cfg.n_btiles),
         "kpad": kpad}
        for i in range(cfg.n_cores)
    ]


def kernel(inputs: np.ndarray, kernel: np.ndarray) -> np.ndarray:
    """CapsuleLayer forward: inputs [16384, 2048] f32, kernel [2048, 160] f32
    -> [16384, 16] f32. Runs SPMD across 8 NeuronCores (batch split 8 ways)."""
    from concourse.bass_utils import run_bass_kernel_spmd

    cfg = BEST
    assert inputs.shape == (cfg.bc * cfg.n_cores, D)
    assert kernel.shape == (D, NCOL)
    if "nc" not in _CACHE:
        _CACHE["nc"] = build(cfg)
    nc = _CACHE["nc"]

    x = np.ascontiguousarray(inputs, dtype=np.float32)
    in_maps = prep_in_maps(x, kernel, cfg)
    res = run_bass_kernel_spmd(nc, in_maps, list(range(cfg.n_cores)))
    outs = []
    for i in range(cfg.n_cores):
        y = res.results[i]["yout"]
        if cfg.yraw:
            # [p, t*16+d] -> [t*128+p, d]
            y = np.ascontiguousarray(
                y.reshape(128, cfg.n_btiles, DCAP).transpose(1, 0, 2)
            ).reshape(cfg.bc, DCAP)
        outs.append(y)
    return np.concatenate(outs, axis=0).astype(np.float32)



# revision 40
# speedup vs baseline: 1.0325x; 1.0325x over previous
"""CapsuleLayer Bass/Tile kernel for TRN2 (one NeuronCore; replicated SPMD x8).

Host-side prep transposes+casts x to fp16 in a per-b-tile layout so the
device does no transposes at all:
  xin[t*128 + p, j*128 + b] = x[t*128 + b, j*128 + p]   (fp16)
Per 128-sample b-tile t, the 16 chunks j are matmul lhsT operands
directly: out[b, c] += sum_p xin_t[p, j*128+b] * kpad[p, j*PADN+c].

kpad (fp16) holds the [2048, 160] kernel as [128, 16*PADN] with col
160:176 of each chunk = 0.1 * sum over capsule blocks, so the first
routing iteration's weighted sum falls out of the matmul.

Routing runs on [128, G*160] group-wide tiles (DVE/ACT/GPSIMD). sqrt is
computed as exp(0.5*ln(x)) so every ACT func (Copy/Ln/Exp) lives in one
activation table set (no LoadActFuncSet thrash).
"""

from dataclasses import dataclass

import numpy as np

import concourse.bacc as bacc
import concourse.tile as tile
from concourse import mybir

NCAP = 10
DCAP = 16
EPS = 1e-7
D = 2048
NCOL = NCAP * DCAP  # 160
NCHUNK = 16  # 2048 / 128


@dataclass
class Cfg:
    n_btiles: int = 16          # 128-sample tiles per core
    group: int = 4              # b-tiles per routing group
    pad_n: int = 176            # kpad columns per chunk (160 hat + 16 s1)
    n_cores: int = 8
    reps: int = 1               # repeat whole pipeline (for slope timing)
    dma_btiles: int = 1         # b-tiles per input DMA
    loop_reps: int = 0          # >0: wrap body in a hardware For_i loop
    group_sizes: str = ""       # e.g. "8,5,3"; overrides group when set
    big_pat: str = "ddpdddpd"   # engine per big routing op (p=Pool, d=DVE)
    pump: int = 8               # routing steps pumped per b-tile emitted
    x_bufs: int = 6
    phat_bufs: int = 8
    hatw_bufs: int = 3
    rt_bufs: int = 3
    sm_bufs: int = 3
    outs_bufs: int = 3
    copy_eng: str = "a"         # PSUM->SBUF copy engine: a=ACT p=Pool, or
                                # a multi-char pattern cycled per b-tile
    small_eng: str = "d"        # engine for squash small muls/stt (d/p)
    red_split: int = 0          # 1: halve big reduces with a Pool add stage
    sub3: int = 0               # 1: subtract max before softmax3 (f16 iter3)
    load_eng: str = "s"         # xin load queue per b-tile: s=SP a=ACT,
                                # multi-char pattern cycled
    store_eng: str = "a"        # yout store queue: a=ACT hwdge, p=Pool swdge
    yraw: int = 0               # 1: store yout as [128, NB*DCAP] (contiguous
                                # per-partition runs; host unshuffles)
    v3_eng: str = "d"           # engine for the final v3 mul (d=DVE p=Pool)

    @property
    def bc(self):
        return self.n_btiles * 128


def prep_x(x_core: np.ndarray, n_btiles: int) -> np.ndarray:
    """[Bc, 2048] f32 -> [Bc, 2048] fp16 laid out [t*128+p, j*128+b]."""
    bc = x_core.shape[0]
    assert x_core.shape == (bc, D) and bc == n_btiles * 128
    a = x_core.reshape(n_btiles, 128, NCHUNK, 128)  # [t, b, j, p]
    a = np.ascontiguousarray(a.transpose(0, 3, 2, 1))  # [t, p, j, b]
    return a.reshape(bc, D).astype(np.float16)


def make_kpad(kernel: np.ndarray, pad_n: int) -> np.ndarray:
    """[2048, 160] f32 -> [128, 16*pad_n] fp16, kpad[p, j*pad_n+c] =
    k[j*128+p, c]; col 160:176 = 0.1 * sum over capsules; rest zero."""
    d, ncol = kernel.shape
    assert (d, ncol) == (D, NCOL)
    kp = np.zeros((NCHUNK, 128, pad_n), dtype=np.float32)
    kj = kernel.reshape(NCHUNK, 128, NCOL)
    kp[:, :, :NCOL] = kj
    kp[:, :, NCOL:NCOL + DCAP] = 0.1 * kj.reshape(NCHUNK, 128, NCAP, DCAP).sum(axis=2)
    out = kp.transpose(1, 0, 2).reshape(128, NCHUNK * pad_n)
    return np.ascontiguousarray(out).astype(np.float16)


def build(cfg: Cfg):
    nc = bacc.Bacc("TRN2", target_bir_lowering=False, debug=False,
                   num_devices=cfg.n_cores)
    f16 = mybir.dt.float16
    f32 = mybir.dt.float32

    NB = cfg.n_btiles
    PADN = cfg.pad_n
    if cfg.group_sizes:
        sizes = [int(s) for s in cfg.group_sizes.split(",")]
    else:
        assert NB % cfg.group == 0
        sizes = [cfg.group] * (NB // cfg.group)
    assert sum(sizes) == NB

    eps_t = nc.alloc_sbuf_tensor("const-eps", [128, 1], f32)
    nc.gpsimd.memset(eps_t.ap(), EPS)
    nc.const_aps.aps[(f32, EPS)] = eps_t.ap()
    nc.all_engine_barrier()

    xin = nc.dram_tensor("xin", [cfg.bc, D], f16, kind="ExternalInput")
    kpad = nc.dram_tensor("kpad", [128, NCHUNK * PADN], f16,
                          kind="ExternalInput")
    # f16 output (host upcasts): halves store traffic; output absmax ~0.8
    # so f16 quantization (~5e-4 rel) is negligible vs the 2e-2 budget
    if cfg.yraw:
        # [p, t*16+d] = out[t*128+p, d]: 256B-contiguous per-partition runs
        # per group store instead of 32B rows (host unshuffles)
        yout = nc.dram_tensor("yout", [128, cfg.n_btiles * DCAP], f16,
                              kind="ExternalOutput")
    else:
        yout = nc.dram_tensor("yout", [cfg.bc, DCAP], f16,
                              kind="ExternalOutput")

    with tile.TileContext(nc) as tc:
        with (
            tc.tile_pool(name="const", bufs=1) as constp,
            tc.tile_pool(name="xt", bufs=cfg.x_bufs) as xtp,
            tc.tile_pool(name="phat", bufs=cfg.phat_bufs, space="PSUM") as phatp,
            tc.tile_pool(name="hatw", bufs=cfg.hatw_bufs) as hatwp,
            tc.tile_pool(name="rt", bufs=cfg.rt_bufs) as rtp,
            tc.tile_pool(name="sm", bufs=cfg.sm_bufs) as smp,
            tc.tile_pool(name="outs", bufs=cfg.outs_bufs) as outsp,
        ):
            # kpad is loop-invariant: load once, outside any timing loop
            kp_t = constp.tile([128, NCHUNK * PADN], f16, tag="kpad")
            nc.sync.dma_start(kp_t[:], kpad[:, :])

            xv = xin[:, :].rearrange("(t p) d -> t p d", p=128)

            DB = cfg.dma_btiles
            x_slices = {}  # i -> (tile, col offset)

            def load_chunk(i0):
                xt = xtp.tile([128, DB * D], f16, tag="xt")
                le = cfg.load_eng[(i0 // DB) % len(cfg.load_eng)]
                eng = nc.scalar if le == "a" else nc.sync
                if DB == 1:
                    eng.dma_start(xt[:], xv[i0])
                else:
                    eng.dma_start(
                        xt[:].rearrange("p (t d) -> p t d", t=DB),
                        xin[:, :].rearrange("(c t p) d -> c p t d",
                                            t=DB, p=128)[i0 // DB],
                    )
                for t in range(DB):
                    x_slices[i0 + t] = (xt, t * D)

            S1C = PADN  # 176 = 11 * 16 per-tile block in hatw

            def big_engines():
                """Engine pattern for the 8 big [128, G*160] ops per group:
                order of use: mul2,red2, mulw2,redw2, mul3,red3, mulw3,redw3."""
                pat = []
                for ch in cfg.big_pat:
                    pat.append(nc.gpsimd if ch == "p" else nc.vector)
                return pat

            def routing_gen(i0, G, gi, hatw):
                """Generator emitting routing ops for one group; yields
                between ops so the driver can interleave groups (wavefront
                order keeps every engine queue head runnable)."""
                if cfg.yraw:
                    yv = yout[:, i0 * DCAP:(i0 + G) * DCAP].rearrange(
                        "p (g d) -> p g d", g=G)
                else:
                    yv = yout[i0 * 128:(i0 + G) * 128, :].rearrange(
                        "(g p) d -> p g d", p=128)
                W = hatw[:].rearrange("p (g n d) -> p g n d", n=NCAP + 1,
                                      d=DCAP)
                H = W[:, :, 0:NCAP, :]
                S1 = W[:, :, NCAP, :]
                eng = big_engines()

                def sqrt_eps(n2, tag):
                    lg = smp.tile([128, G], f32, tag=f"lg{tag}")
                    nc.scalar.activation(lg[:], n2,
                                         mybir.ActivationFunctionType.Ln,
                                         bias=EPS)
                    sr = smp.tile([128, G], f32, tag=f"sr{tag}")
                    nc.scalar.activation(sr[:], lg[:],
                                         mybir.ActivationFunctionType.Exp,
                                         scale=0.5)
                    return sr

                sm_eng = nc.gpsimd if cfg.small_eng == "p" else nc.vector

                def squash_steps(su, sdt, r, tag, out):
                    """out[0] = comb scale c s.t. v = c*su*(r or 1)."""
                    sq = smp.tile([128, G * DCAP], sdt, tag=f"sq{tag}")
                    nc.scalar.activation(sq[:], su,
                                         mybir.ActivationFunctionType.Square)
                    yield
                    m2 = smp.tile([128, G], f32, tag=f"m2{tag}")
                    nc.vector.tensor_reduce(
                        m2[:], sq[:].rearrange("p (g d) -> p g d", g=G),
                        axis=mybir.AxisListType.X, op=mybir.AluOpType.add)
                    yield
                    if r is not None:
                        rr = smp.tile([128, G], f32, tag=f"rr{tag}")
                        sm_eng.tensor_mul(rr[:], r, r)
                        n2 = smp.tile([128, G], f32, tag=f"n2{tag}")
                        sm_eng.tensor_mul(n2[:], m2[:], rr[:])
                        yield
                    else:
                        n2 = m2
                    sr = sqrt_eps(n2[:], tag)
                    yield
                    den = smp.tile([128, G], f32, tag=f"den{tag}")
                    nc.vector.scalar_tensor_tensor(
                        den[:], n2[:], 1.0, sr[:],
                        op0=mybir.AluOpType.add, op1=mybir.AluOpType.mult)
                    rec = smp.tile([128, G], f32, tag=f"rec{tag}")
                    nc.vector.reciprocal(rec[:], den[:])
                    yield
                    sc = smp.tile([128, G], f32, tag=f"sc{tag}")
                    sm_eng.tensor_mul(sc[:], n2[:], rec[:])
                    out[1] = sc
                    if r is not None:
                        comb = smp.tile([128, G], f32, tag=f"comb{tag}")
                        sm_eng.tensor_mul(comb[:], sc[:], r)
                        out[0] = comb
                    else:
                        out[0] = sc
                    yield

                def dots_steps(src_gd, e0, e1, tag, out):
                    """r[g,n] = sum_d H[g,n,d]*src[g,d] -> [128, G*NCAP] f32"""
                    tmp = rtp.tile([128, G * NCOL], f16, tag=f"dt{tag}")
                    bc = src_gd.unsqueeze(2).broadcast_to((128, G, NCAP, DCAP))
                    e0.tensor_mul(
                        tmp[:].rearrange("p (g n d) -> p g n d", g=G, n=NCAP),
                        H, bc)
                    yield
                    v = tmp[:].rearrange("p (g n d) -> p g n d", g=G, n=NCAP)
                    if cfg.red_split:
                        half = rtp.tile([128, G * NCAP * (DCAP // 2)], f16,
                                        tag=f"dh{tag}")
                        hv = half[:].rearrange("p (g n d) -> p g n d",
                                               g=G, n=NCAP)
                        with nc.allow_low_precision(reason="f16 pair sums"):
                            nc.gpsimd.tensor_add(hv, v[:, :, :, 0:DCAP // 2],
                                                 v[:, :, :, DCAP // 2:DCAP])
                        v = hv
                        yield
                    o = rtp.tile([128, G * NCAP], f32, tag=f"dr{tag}")
                    e1.tensor_reduce(
                        o[:], v, axis=mybir.AxisListType.X,
                        op=mybir.AluOpType.add)
                    out[0] = o
                    yield

                def wsum_steps(e_gn, edt, e0, e1, tag, out):
                    """su[g,d] = sum_n H[g,n,d]*e[g,n] -> [128, G*DCAP] edt"""
                    tmp = rtp.tile([128, G * NCOL], edt, tag=f"wt{tag}")
                    bc = e_gn.unsqueeze(3).broadcast_to((128, G, NCAP, DCAP))
                    e0.tensor_mul(
                        tmp[:].rearrange("p (g n d) -> p g n d", g=G, n=NCAP),
                        H, bc)
                    yield
                    v = tmp[:].rearrange("p (g n d) -> p g d n", g=G, n=NCAP)
                    if cfg.red_split:
                        half = rtp.tile([128, G * (NCAP // 2) * DCAP], edt,
                                        tag=f"wh{tag}")
                        hv = half[:].rearrange("p (g n d) -> p g d n",
                                               g=G, n=NCAP // 2)
                        with nc.allow_low_precision(reason="pair sums; rel "
                                                    "tol 2e-2"):
                            nc.gpsimd.tensor_add(
                                hv.rearrange("p g d n -> p g n d"),
                                tmp[:].rearrange("p (g n d) -> p g n d",
                                                 g=G, n=NCAP)[:, :, 0:5, :],
                                tmp[:].rearrange("p (g n d) -> p g n d",
                                                 g=G, n=NCAP)[:, :, 5:10, :])
                        v = hv
                        yield
                    o = rtp.tile([128, G * DCAP], edt, tag=f"ws{tag}")
                    with nc.allow_low_precision(reason="f16 weighted sum fits "
                                                "range; rel tol 2e-2"):
                        e1.tensor_reduce(
                            o[:], v, axis=mybir.AxisListType.X,
                            op=mybir.AluOpType.add)
                    out[0] = o
                    yield

                def softmax_steps(t_gn, edt, tag, out):
                    """e = exp(t) [128, G*NCAP]; r = 1/sum_n e [128, G]"""
                    e = rtp.tile([128, G * NCAP], edt, tag=f"e{tag}")
                    nc.scalar.activation(e[:], t_gn,
                                         mybir.ActivationFunctionType.Exp)
                    yield
                    se = smp.tile([128, G], f32, tag=f"se{tag}")
                    nc.vector.tensor_reduce(
                        se[:], e[:].rearrange("p (g n) -> p g n", g=G),
                        axis=mybir.AxisListType.X, op=mybir.AluOpType.add)
                    ri = smp.tile([128, G], f32, tag=f"ri{tag}")
                    nc.vector.reciprocal(ri[:], se[:])
                    out[0] = (e, ri)
                    yield

                gv = lambda ap: ap.rearrange("p (g d) -> p g d", g=G)
                nv = lambda ap: ap.rearrange("p (g n) -> p g n", g=G)

                # iter 1: s1 (pre-scaled mean) came from the matmul (fp16)
                c1, r2o = [None, None], [None]
                yield from squash_steps(S1, f16, None, "1", c1)
                yield from dots_steps(S1, eng[0], eng[1], "2", r2o)
                t2 = rtp.tile([128, G * NCAP], f32, tag="t2")
                nc.gpsimd.tensor_mul(
                    nv(t2[:]), nv(r2o[0][:]),
                    c1[0][:].unsqueeze(2).broadcast_to((128, G, NCAP)))
                yield

                # iter 2
                sm2 = [None]
                yield from softmax_steps(t2[:], f16, "2", sm2)
                e2, r2i = sm2[0]
                s2o = [None]
                yield from wsum_steps(nv(e2[:]), f16, eng[2], eng[3], "2", s2o)
                s2u = s2o[0]
                # dots run on the UNnormalized f16 s2u; the softmax norm r2i
                # is folded into the comb scale (c2[0] = sc*r2i) at t3.
                c2, r3o = [None, None], [None]
                yield from squash_steps(s2u[:], f32, r2i[:], "2", c2)
                yield from dots_steps(gv(s2u[:]), eng[4], eng[5], "3", r3o)
                t3 = rtp.tile([128, G * NCAP], f32, tag="t3")
                nc.gpsimd.tensor_mul(
                    nv(t3[:]), nv(r3o[0][:]),
                    c2[0][:].unsqueeze(2).broadcast_to((128, G, NCAP)))
                nc.gpsimd.tensor_add(t3[:], t3[:], t2[:])
                yield

                # iter 3: b3 logits reach ~16 so exp(b3) would overflow f16.
                # sub3: subtract the per-sample max (softmax-invariant) so
                # e3 <= 1 and the whole iteration runs in f16; else f32.
                if cfg.sub3:
                    m3 = smp.tile([128, G], f32, tag="m3")
                    nc.vector.tensor_reduce(
                        m3[:], nv(t3[:]), axis=mybir.AxisListType.X,
                        op=mybir.AluOpType.max)
                    yield
                    t3s = rtp.tile([128, G * NCAP], f16, tag="t3s")
                    with nc.allow_low_precision(reason="shifted logits are "
                                                "small"):
                        nc.gpsimd.tensor_sub(
                            nv(t3s[:]), nv(t3[:]),
                            m3[:].unsqueeze(2).broadcast_to((128, G, NCAP)))
                    yield
                    t3in, dt3 = t3s, f16
                else:
                    t3in, dt3 = t3, f32
                sm3 = [None]
                yield from softmax_steps(t3in[:], dt3, "3", sm3)
                e3, r3i = sm3[0]
                s3o = [None]
                yield from wsum_steps(nv(e3[:]), dt3, eng[6], eng[7], "3", s3o)
                s3u = s3o[0]
                c3 = [None, None]
                yield from squash_steps(s3u[:], f32, r3i[:], "3", c3)
                v3 = outsp.tile([128, G * DCAP], f16, tag="v3")
                # v3 feeds only the async store: latency-tolerant, so run it
                # on Pool (idle) instead of saturated DVE when v3_eng=p
                v3_e = nc.gpsimd if cfg.v3_eng == "p" else nc.vector
                with nc.allow_low_precision(reason="f16 output store"):
                    v3_e.tensor_mul(
                        gv(v3[:]), gv(s3u[:]),
                        c3[0][:].unsqueeze(2).broadcast_to((128, G, DCAP)))
                # store off the SP queue so next-iteration xin loads are
                # never stuck behind routing-dependent stores
                st_eng = nc.gpsimd if cfg.store_eng == "p" else nc.scalar
                st_eng.dma_start(
                    yv,
                    v3[:].rearrange("p (g d) -> p g d", g=G))
                yield

            active = []

            def pump(k):
                for _ in range(k):
                    for gen in list(active):
                        try:
                            next(gen)
                        except StopIteration:
                            active.remove(gen)

            def run_all():
                # NOTE: no drain at the end — generators stay active across
                # unrolled reps so the last group's routing tail is emitted
                # interleaved with the next rep's loads/copies (otherwise
                # every engine queue serializes behind the full tail at each
                # rep boundary). drain_all() must run before the For_i
                # boundary.
                i0 = 0
                for gi, G in enumerate(sizes):
                    hatw = hatwp.tile([128, G * S1C], f16, tag="hatw")
                    for g in range(G):
                        i = i0 + g
                        if i % DB == 0:
                            load_chunk(i)
                        xt, off = x_slices.pop(i)
                        ph = phatp.tile([128, PADN], f32, tag="phat")
                        for j in range(NCHUNK):
                            nc.tensor.matmul(
                                ph[:],
                                xt[:, off + j * 128:off + (j + 1) * 128],
                                kp_t[:, j * PADN:(j + 1) * PADN],
                                start=(j == 0),
                                stop=(j == NCHUNK - 1),
                            )
                        ce = cfg.copy_eng[i % len(cfg.copy_eng)]
                        dst = hatw[:, g * S1C:(g + 1) * S1C]
                        if ce == "p":
                            nc.gpsimd.tensor_copy(dst, ph[:, :S1C])
                        elif ce == "v":
                            nc.vector.tensor_copy(dst, ph[:, :S1C])
                        else:
                            nc.scalar.copy(dst, ph[:, :S1C])
                        pump(cfg.pump)
                    active.append(routing_gen(i0, G, gi, hatw))
                    i0 += G

            def drain_all():
                while active:
                    pump(1)

            if cfg.loop_reps > 0:
                # hardware loop around an unrolled body: the For_i barrier
                # cost amortizes over cfg.reps unrolled pipeline repeats
                with tc.For_i(0, cfg.loop_reps, 1,
                              hint_engines=(mybir.EngineType.PE,)):
                    for _rep in range(cfg.reps):
                        run_all()
                    drain_all()
            else:
                for _rep in range(cfg.reps):
                    run_all()
                drain_all()

    nc.compile()
    _unify_act_tables(nc)
    return nc


def _unify_act_tables(nc):
    """Replace the greedy per-func LoadActFuncSet placement with a single
    load of a set containing every activation func this kernel uses
    (Copy/Identity/Square/Exp/Ln all live in natural_log_exp_and_others).
    The greedy pass alternates exp_and_others <-> natural_log per squash,
    costing ~1.3us per reload on the ACT engine."""
    from concourse.hw_specs import get_activation_tables

    AF = mybir.ActivationFunctionType
    needed = {AF.Copy, AF.Identity, AF.Square, AF.Exp, AF.Ln}
    tables = list(get_activation_tables(nc.m.arch).items())
    combined_id = None
    for idx, (name, funcs) in enumerate(tables):
        if needed <= funcs:
            combined_id = idx
            break
    if combined_id is None:
        return  # no single set covers us; keep the pass's placement

    fn = nc.m.functions[0]
    blocks = list(fn.blocks)
    kept = None
    for blk in blocks:
        insts = list(blk.instructions)
        out = []
        for i in insts:
            if isinstance(i, mybir.InstLoadActFuncSet):
                if kept is None:
                    i.act_func_set_id = combined_id
                    kept = i
                continue  # drop (re-inserted once below)
            out.append(i)
        if len(out) != len(insts):
            blk.instructions[:] = out
    if kept is None:
        return
    # Place the single load in the entry block (before its terminator) so
    # it dominates every activation, including those inside For_i loops.
    entry = blocks[0]
    insts = list(entry.instructions)
    pos = len(insts)
    while pos > 0 and insts[pos - 1].opcode in ("UnconditionalBranch",
                                                "ConditionalBranch"):
        pos -= 1
    entry.instructions[:] = insts[:pos] + [kept] + insts[pos:]


# ---------------- numpy reference (per-core) ----------------

def ref_numpy(x: np.ndarray, kernel: np.ndarray) -> np.ndarray:
    b = x.shape[0]
    hat = (x @ kernel).reshape(b, NCAP, DCAP)
    logits = np.zeros((b, NCAP, 1), dtype=x.dtype)
    out = None
    for _ in range(3):
        ex = np.exp(logits - logits.max(axis=1, keepdims=True))
        c = ex / ex.sum(axis=1, keepdims=True)
        s = (c * hat).sum(axis=1, keepdims=True)
        s2 = np.square(s).sum(axis=-1, keepdims=True)
        out = s2 / (1.0 + s2) / np.sqrt(s2 + EPS) * s
        logits = logits + np.einsum("bnd,bd->bn", hat, out[:, 0, :])[:, :, None]
    return out[:, 0, :]


# ---------------- public entry point ----------------

_CACHE = {}

BEST = Cfg(n_btiles=16, group_sizes="8,8", big_pat="dddddddd", yraw=1)


def prep_in_maps(x: np.ndarray, kern: np.ndarray, cfg: Cfg):
    kpad = make_kpad(np.asarray(kern, dtype=np.float32), cfg.pad_n)
    return [
        {"xin": prep_x(x[i * cfg.bc:(i + 1) * cfg.bc], cfg.n_btiles),
         "kpad": kpad}
        for i in range(cfg.n_cores)
    ]


def kernel(inputs: np.ndarray, kernel: np.ndarray) -> np.ndarray:
    """CapsuleLayer forward: inputs [16384, 2048] f32, kernel [2048, 160] f32
    -> [16384, 16] f32. Runs SPMD across 8 NeuronCores (batch split 8 ways)."""
    from concourse.bass_utils import run_bass_kernel_spmd

    cfg = BEST
    assert inputs.shape == (cfg.bc * cfg.n_cores, D)
    assert kernel.shape == (D, NCOL)
    if "nc" not in _CACHE:
        _CACHE["nc"] = build(cfg)
    nc = _CACHE["nc"]

    x = np.ascontiguousarray(inputs, dtype=np.float32)
    in_maps = prep_in_maps(x, kernel, cfg)
    res = run_bass_kernel_spmd(nc, in_maps, list(range(cfg.n_cores)))
    outs = []
    for i in range(cfg.n_cores):
        y = res.results[i]["yout"]
        if cfg.yraw:
            # [p, t*16+d] -> [t*128+p, d]
            y = np.ascontiguousarray(
                y.reshape(128, cfg.n_btiles, DCAP).transpose(1, 0, 2)
            ).reshape(cfg.bc, DCAP)
        outs.append(y)
    return np.concatenate(outs, axis=0).astype(np.float32)



# revision 46
# speedup vs baseline: 1.0545x; 1.0214x over previous
"""CapsuleLayer Bass/Tile kernel for TRN2 (one NeuronCore; replicated SPMD x8).

Host-side prep transposes+casts x to fp16 in a per-b-tile layout so the
device does no transposes at all:
  xin[t*128 + p, j*128 + b] = x[t*128 + b, j*128 + p]   (fp16)
Per 128-sample b-tile t, the 16 chunks j are matmul lhsT operands
directly: out[b, c] += sum_p xin_t[p, j*128+b] * kpad[p, j*PADN+c].

kpad (fp16) holds the [2048, 160] kernel as [128, 16*PADN] with col
160:176 of each chunk = 0.1 * sum over capsule blocks, so the first
routing iteration's weighted sum falls out of the matmul.

Routing runs on [128, G*160] group-wide tiles (DVE/ACT/GPSIMD). sqrt is
computed as exp(0.5*ln(x)) so every ACT func (Copy/Ln/Exp) lives in one
activation table set (no LoadActFuncSet thrash).
"""

from dataclasses import dataclass

import numpy as np

import concourse.bacc as bacc
import concourse.tile as tile
from concourse import mybir

NCAP = 10
DCAP = 16
EPS = 1e-7
D = 2048
NCOL = NCAP * DCAP  # 160
NCHUNK = 16  # 2048 / 128


@dataclass
class Cfg:
    n_btiles: int = 16          # 128-sample tiles per core
    group: int = 4              # b-tiles per routing group
    pad_n: int = 176            # kpad columns per chunk (160 hat + 16 s1)
    n_cores: int = 8
    reps: int = 1               # repeat whole pipeline (for slope timing)
    dma_btiles: int = 1         # b-tiles per input DMA
    loop_reps: int = 0          # >0: wrap body in a hardware For_i loop
    group_sizes: str = ""       # e.g. "8,5,3"; overrides group when set
    big_pat: str = "ddpdddpd"   # engine per big routing op (p=Pool, d=DVE)
    pump: int = 8               # routing steps pumped per b-tile emitted
    x_bufs: int = 6
    phat_bufs: int = 8
    hatw_bufs: int = 3
    rt_bufs: int = 3
    sm_bufs: int = 3
    outs_bufs: int = 3
    copy_eng: str = "a"         # PSUM->SBUF copy engine: a=ACT p=Pool, or
                                # a multi-char pattern cycled per b-tile
    small_eng: str = "d"        # engine for squash small muls/stt (d/p)
    red_split: int = 0          # 1: halve big reduces with a Pool add stage
    sub3: int = 0               # 1: subtract max before softmax3 (f16 iter3)
    load_eng: str = "s"         # xin load queue per b-tile: s=SP a=ACT,
                                # multi-char pattern cycled
    store_eng: str = "a"        # yout store queue: a=ACT hwdge, p=Pool swdge
    yraw: int = 0               # 1: store yout as [128, NB*DCAP] (contiguous
                                # per-partition runs; host unshuffles)
    v3_eng: str = "d"           # engine for the final v3 mul (d=DVE p=Pool)
    drain_per_rep: int = 0      # 1: drain routing gens at end of each rep
    tree_red: int = 1           # 1: f16 add-tree (2x DVE) instead of 1x
                                # tensor_reduce where ranges allow

    @property
    def bc(self):
        return self.n_btiles * 128


def prep_x(x_core: np.ndarray, n_btiles: int) -> np.ndarray:
    """[Bc, 2048] f32 -> [Bc, 2048] fp16 laid out [t*128+p, j*128+b]."""
    bc = x_core.shape[0]
    assert x_core.shape == (bc, D) and bc == n_btiles * 128
    a = x_core.reshape(n_btiles, 128, NCHUNK, 128)  # [t, b, j, p]
    a = np.ascontiguousarray(a.transpose(0, 3, 2, 1))  # [t, p, j, b]
    return a.reshape(bc, D).astype(np.float16)


def make_kpad(kernel: np.ndarray, pad_n: int) -> np.ndarray:
    """[2048, 160] f32 -> [128, 16*pad_n] fp16, kpad[p, j*pad_n+c] =
    k[j*128+p, c]; col 160:176 = 0.1 * sum over capsules; rest zero."""
    d, ncol = kernel.shape
    assert (d, ncol) == (D, NCOL)
    kp = np.zeros((NCHUNK, 128, pad_n), dtype=np.float32)
    kj = kernel.reshape(NCHUNK, 128, NCOL)
    kp[:, :, :NCOL] = kj
    kp[:, :, NCOL:NCOL + DCAP] = 0.1 * kj.reshape(NCHUNK, 128, NCAP, DCAP).sum(axis=2)
    out = kp.transpose(1, 0, 2).reshape(128, NCHUNK * pad_n)
    return np.ascontiguousarray(out).astype(np.float16)


def build(cfg: Cfg):
    nc = bacc.Bacc("TRN2", target_bir_lowering=False, debug=False,
                   num_devices=cfg.n_cores)
    f16 = mybir.dt.float16
    f32 = mybir.dt.float32

    NB = cfg.n_btiles
    PADN = cfg.pad_n
    if cfg.group_sizes:
        sizes = [int(s) for s in cfg.group_sizes.split(",")]
    else:
        assert NB % cfg.group == 0
        sizes = [cfg.group] * (NB // cfg.group)
    assert sum(sizes) == NB

    eps_t = nc.alloc_sbuf_tensor("const-eps", [128, 1], f32)
    nc.gpsimd.memset(eps_t.ap(), EPS)
    nc.const_aps.aps[(f32, EPS)] = eps_t.ap()
    nc.all_engine_barrier()

    xin = nc.dram_tensor("xin", [cfg.bc, D], f16, kind="ExternalInput")
    kpad = nc.dram_tensor("kpad", [128, NCHUNK * PADN], f16,
                          kind="ExternalInput")
    # f16 output (host upcasts): halves store traffic; output absmax ~0.8
    # so f16 quantization (~5e-4 rel) is negligible vs the 2e-2 budget
    if cfg.yraw:
        # [p, t*16+d] = out[t*128+p, d]: 256B-contiguous per-partition runs
        # per group store instead of 32B rows (host unshuffles)
        yout = nc.dram_tensor("yout", [128, cfg.n_btiles * DCAP], f16,
                              kind="ExternalOutput")
    else:
        yout = nc.dram_tensor("yout", [cfg.bc, DCAP], f16,
                              kind="ExternalOutput")

    with tile.TileContext(nc) as tc:
        with (
            tc.tile_pool(name="const", bufs=1) as constp,
            tc.tile_pool(name="xt", bufs=cfg.x_bufs) as xtp,
            tc.tile_pool(name="phat", bufs=cfg.phat_bufs, space="PSUM") as phatp,
            tc.tile_pool(name="hatw", bufs=cfg.hatw_bufs) as hatwp,
            tc.tile_pool(name="rt", bufs=cfg.rt_bufs) as rtp,
            tc.tile_pool(name="sm", bufs=cfg.sm_bufs) as smp,
            tc.tile_pool(name="outs", bufs=cfg.outs_bufs) as outsp,
        ):
            # kpad is loop-invariant: load once, outside any timing loop
            kp_t = constp.tile([128, NCHUNK * PADN], f16, tag="kpad")
            nc.sync.dma_start(kp_t[:], kpad[:, :])

            xv = xin[:, :].rearrange("(t p) d -> t p d", p=128)

            DB = cfg.dma_btiles
            x_slices = {}  # i -> (tile, col offset)

            def load_chunk(i0):
                xt = xtp.tile([128, DB * D], f16, tag="xt")
                le = cfg.load_eng[(i0 // DB) % len(cfg.load_eng)]
                eng = nc.scalar if le == "a" else nc.sync
                if DB == 1:
                    eng.dma_start(xt[:], xv[i0])
                else:
                    eng.dma_start(
                        xt[:].rearrange("p (t d) -> p t d", t=DB),
                        xin[:, :].rearrange("(c t p) d -> c p t d",
                                            t=DB, p=128)[i0 // DB],
                    )
                for t in range(DB):
                    x_slices[i0 + t] = (xt, t * D)

            S1C = PADN  # 176 = 11 * 16 per-tile block in hatw

            def big_engines():
                """Engine pattern for the 8 big [128, G*160] ops per group:
                order of use: mul2,red2, mulw2,redw2, mul3,red3, mulw3,redw3."""
                pat = []
                for ch in cfg.big_pat:
                    pat.append(nc.gpsimd if ch == "p" else nc.vector)
                return pat

            def routing_gen(i0, G, gi, hatw):
                """Generator emitting routing ops for one group; yields
                between ops so the driver can interleave groups (wavefront
                order keeps every engine queue head runnable)."""
                if cfg.yraw:
                    yv = yout[:, i0 * DCAP:(i0 + G) * DCAP].rearrange(
                        "p (g d) -> p g d", g=G)
                else:
                    yv = yout[i0 * 128:(i0 + G) * 128, :].rearrange(
                        "(g p) d -> p g d", p=128)
                W = hatw[:].rearrange("p (g n d) -> p g n d", n=NCAP + 1,
                                      d=DCAP)
                H = W[:, :, 0:NCAP, :]
                S1 = W[:, :, NCAP, :]
                eng = big_engines()

                def sqrt_eps(n2, tag):
                    lg = smp.tile([128, G], f32, tag=f"lg{tag}")
                    nc.scalar.activation(lg[:], n2,
                                         mybir.ActivationFunctionType.Ln,
                                         bias=EPS)
                    sr = smp.tile([128, G], f32, tag=f"sr{tag}")
                    nc.scalar.activation(sr[:], lg[:],
                                         mybir.ActivationFunctionType.Exp,
                                         scale=0.5)
                    return sr

                sm_eng = nc.gpsimd if cfg.small_eng == "p" else nc.vector

                def squash_steps(su, sdt, r, tag, out):
                    """out[0] = comb scale c s.t. v = c*su*(r or 1)."""
                    sq = smp.tile([128, G * DCAP], sdt, tag=f"sq{tag}")
                    nc.scalar.activation(sq[:], su,
                                         mybir.ActivationFunctionType.Square)
                    yield
                    m2 = smp.tile([128, G], f32, tag=f"m2{tag}")
                    nc.vector.tensor_reduce(
                        m2[:], sq[:].rearrange("p (g d) -> p g d", g=G),
                        axis=mybir.AxisListType.X, op=mybir.AluOpType.add)
                    yield
                    if r is not None:
                        rr = smp.tile([128, G], f32, tag=f"rr{tag}")
                        sm_eng.tensor_mul(rr[:], r, r)
                        n2 = smp.tile([128, G], f32, tag=f"n2{tag}")
                        sm_eng.tensor_mul(n2[:], m2[:], rr[:])
                        yield
                    else:
                        n2 = m2
                    sr = sqrt_eps(n2[:], tag)
                    yield
                    den = smp.tile([128, G], f32, tag=f"den{tag}")
                    nc.vector.scalar_tensor_tensor(
                        den[:], n2[:], 1.0, sr[:],
                        op0=mybir.AluOpType.add, op1=mybir.AluOpType.mult)
                    rec = smp.tile([128, G], f32, tag=f"rec{tag}")
                    nc.vector.reciprocal(rec[:], den[:])
                    yield
                    sc = smp.tile([128, G], f32, tag=f"sc{tag}")
                    sm_eng.tensor_mul(sc[:], n2[:], rec[:])
                    out[1] = sc
                    if r is not None:
                        comb = smp.tile([128, G], f32, tag=f"comb{tag}")
                        sm_eng.tensor_mul(comb[:], sc[:], r)
                        out[0] = comb
                    else:
                        out[0] = sc
                    yield

                def dots_steps(src_gd, e0, e1, tag, out):
                    """r[g,n] = sum_d H[g,n,d]*src[g,d] -> [128, G*NCAP] f32"""
                    tmp = rtp.tile([128, G * NCOL], f16, tag=f"dt{tag}")
                    bc = src_gd.unsqueeze(2).broadcast_to((128, G, NCAP, DCAP))
                    e0.tensor_mul(
                        tmp[:].rearrange("p (g n d) -> p g n d", g=G, n=NCAP),
                        H, bc)
                    yield
                    v = tmp[:].rearrange("p (g n d) -> p g n d", g=G, n=NCAP)
                    if cfg.tree_red and tag == "2":
                        # iter-1 dots products are tiny: f16 add-tree runs at
                        # the 2x DVE rate the 1x tensor_reduce never gets
                        w = DCAP
                        with nc.allow_low_precision(reason="f16 tree sums; "
                                                    "small products"):
                            while w > 2:
                                h = rtp.tile([128, G * NCAP * (w // 2)], f16,
                                             tag=f"dt{tag}w{w}")
                                hv = h[:].rearrange("p (g n d) -> p g n d",
                                                    g=G, n=NCAP)
                                nc.vector.tensor_add(
                                    hv, v[:, :, :, 0:w // 2],
                                    v[:, :, :, w // 2:w])
                                v = hv
                                w //= 2
                                yield
                        o = rtp.tile([128, G * NCAP], f32, tag=f"dr{tag}")
                        nc.vector.tensor_add(
                            o[:].rearrange("p (g n) -> p g n", g=G)
                            .unsqueeze(3),
                            v[:, :, :, 0:1], v[:, :, :, 1:2])
                        out[0] = o
                        yield
                        return
                    o = rtp.tile([128, G * NCAP], f32, tag=f"dr{tag}")
                    e1.tensor_reduce(
                        o[:], v, axis=mybir.AxisListType.X,
                        op=mybir.AluOpType.add)
                    out[0] = o
                    yield

                def wsum_steps(e_gn, edt, e0, e1, tag, out):
                    """su[g,d] = sum_n H[g,n,d]*e[g,n] -> [128, G*DCAP] edt"""
                    tmp = rtp.tile([128, G * NCOL], edt, tag=f"wt{tag}")
                    bc = e_gn.unsqueeze(3).broadcast_to((128, G, NCAP, DCAP))
                    e0.tensor_mul(
                        tmp[:].rearrange("p (g n d) -> p g n d", g=G, n=NCAP),
                        H, bc)
                    yield
                    vt = tmp[:].rearrange("p (g n d) -> p g n d", g=G, n=NCAP)
                    if cfg.tree_red and edt == f16:
                        # f16 add-tree over n (all packed-last, 2x DVE rate;
                        # sidesteps the strided n-reduce entirely)
                        with nc.allow_low_precision(reason="f16 tree sums "
                                                    "fit range"):
                            b1 = rtp.tile([128, G * 5 * DCAP], f16,
                                          tag=f"wb1{tag}")
                            v1 = b1[:].rearrange("p (g n d) -> p g n d",
                                                 g=G, n=5)
                            nc.vector.tensor_add(v1, vt[:, :, 0:5, :],
                                                 vt[:, :, 5:10, :])
                            yield
                            b2 = rtp.tile([128, G * 2 * DCAP], f16,
                                          tag=f"wb2{tag}")
                            v2 = b2[:].rearrange("p (g n d) -> p g n d",
                                                 g=G, n=2)
                            nc.vector.tensor_add(v2, v1[:, :, 0:2, :],
                                                 v1[:, :, 2:4, :])
                            yield
                            b3 = rtp.tile([128, G * DCAP], f16,
                                          tag=f"wb3{tag}")
                            v3t = b3[:].rearrange("p (g d) -> p g d", g=G)
                            nc.vector.tensor_add(
                                v3t.unsqueeze(2),
                                v2[:, :, 0:1, :], v2[:, :, 1:2, :])
                            yield
                            o = rtp.tile([128, G * DCAP], f16, tag=f"ws{tag}")
                            nc.vector.tensor_add(
                                o[:].rearrange("p (g d) -> p g d", g=G)
                                .unsqueeze(2),
                                v3t.unsqueeze(2), v1[:, :, 4:5, :])
                        out[0] = o
                        yield
                        return
                    o = rtp.tile([128, G * DCAP], edt, tag=f"ws{tag}")
                    with nc.allow_low_precision(reason="f16 weighted sum fits "
                                                "range; rel tol 2e-2"):
                        e1.tensor_reduce(
                            o[:], vt.rearrange("p g n d -> p g d n"),
                            axis=mybir.AxisListType.X, op=mybir.AluOpType.add)
                    out[0] = o
                    yield

                def softmax_steps(t_gn, edt, tag, out):
                    """e = exp(t) [128, G*NCAP]; r = 1/sum_n e [128, G]"""
                    e = rtp.tile([128, G * NCAP], edt, tag=f"e{tag}")
                    nc.scalar.activation(e[:], t_gn,
                                         mybir.ActivationFunctionType.Exp)
                    yield
                    se = smp.tile([128, G], f32, tag=f"se{tag}")
                    nc.vector.tensor_reduce(
                        se[:], e[:].rearrange("p (g n) -> p g n", g=G),
                        axis=mybir.AxisListType.X, op=mybir.AluOpType.add)
                    ri = smp.tile([128, G], f32, tag=f"ri{tag}")
                    nc.vector.reciprocal(ri[:], se[:])
                    out[0] = (e, ri)
                    yield

                gv = lambda ap: ap.rearrange("p (g d) -> p g d", g=G)
                nv = lambda ap: ap.rearrange("p (g n) -> p g n", g=G)

                # iter 1: s1 (pre-scaled mean) came from the matmul (fp16)
                c1, r2o = [None, None], [None]
                yield from squash_steps(S1, f16, None, "1", c1)
                yield from dots_steps(S1, eng[0], eng[1], "2", r2o)
                t2 = rtp.tile([128, G * NCAP], f32, tag="t2")
                nc.gpsimd.tensor_mul(
                    nv(t2[:]), nv(r2o[0][:]),
                    c1[0][:].unsqueeze(2).broadcast_to((128, G, NCAP)))
                yield

                # iter 2
                sm2 = [None]
                yield from softmax_steps(t2[:], f16, "2", sm2)
                e2, r2i = sm2[0]
                s2o = [None]
                yield from wsum_steps(nv(e2[:]), f16, eng[2], eng[3], "2", s2o)
                s2u = s2o[0]
                # dots run on the UNnormalized f16 s2u; the softmax norm r2i
                # is folded into the comb scale (c2[0] = sc*r2i) at t3.
                c2, r3o = [None, None], [None]
                yield from squash_steps(s2u[:], f32, r2i[:], "2", c2)
                yield from dots_steps(gv(s2u[:]), eng[4], eng[5], "3", r3o)
                t3 = rtp.tile([128, G * NCAP], f32, tag="t3")
                nc.gpsimd.tensor_mul(
                    nv(t3[:]), nv(r3o[0][:]),
                    c2[0][:].unsqueeze(2).broadcast_to((128, G, NCAP)))
                nc.gpsimd.tensor_add(t3[:], t3[:], t2[:])
                yield

                # iter 3: b3 logits reach ~16 so exp(b3) would overflow f16.
                # sub3: subtract the per-sample max (softmax-invariant) so
                # e3 <= 1 and the whole iteration runs in f16; else f32.
                if cfg.sub3:
                    m3 = smp.tile([128, G], f32, tag="m3")
                    nc.vector.tensor_reduce(
                        m3[:], nv(t3[:]), axis=mybir.AxisListType.X,
                        op=mybir.AluOpType.max)
                    yield
                    t3s = rtp.tile([128, G * NCAP], f16, tag="t3s")
                    with nc.allow_low_precision(reason="shifted logits are "
                                                "small"):
                        nc.gpsimd.tensor_sub(
                            nv(t3s[:]), nv(t3[:]),
                            m3[:].unsqueeze(2).broadcast_to((128, G, NCAP)))
                    yield
                    t3in, dt3 = t3s, f16
                else:
                    t3in, dt3 = t3, f32
                sm3 = [None]
                yield from softmax_steps(t3in[:], dt3, "3", sm3)
                e3, r3i = sm3[0]
                s3o = [None]
                yield from wsum_steps(nv(e3[:]), dt3, eng[6], eng[7], "3", s3o)
                s3u = s3o[0]
                c3 = [None, None]
                yield from squash_steps(s3u[:], f32, r3i[:], "3", c3)
                v3 = outsp.tile([128, G * DCAP], f16, tag="v3")
                # v3 feeds only the async store: latency-tolerant, so run it
                # on Pool (idle) instead of saturated DVE when v3_eng=p
                v3_e = nc.gpsimd if cfg.v3_eng == "p" else nc.vector
                with nc.allow_low_precision(reason="f16 output store"):
                    v3_e.tensor_mul(
                        gv(v3[:]), gv(s3u[:]),
                        c3[0][:].unsqueeze(2).broadcast_to((128, G, DCAP)))
                # store off the SP queue so next-iteration xin loads are
                # never stuck behind routing-dependent stores
                st_eng = nc.gpsimd if cfg.store_eng == "p" else nc.scalar
                st_eng.dma_start(
                    yv,
                    v3[:].rearrange("p (g d) -> p g d", g=G))
                yield

            active = []

            def pump(k):
                for _ in range(k):
                    for gen in list(active):
                        try:
                            next(gen)
                        except StopIteration:
                            active.remove(gen)

            def run_all():
                # NOTE: no drain at the end — generators stay active across
                # unrolled reps so the last group's routing tail is emitted
                # interleaved with the next rep's loads/copies (otherwise
                # every engine queue serializes behind the full tail at each
                # rep boundary). drain_all() must run before the For_i
                # boundary.
                i0 = 0
                for gi, G in enumerate(sizes):
                    hatw = hatwp.tile([128, G * S1C], f16, tag="hatw")
                    for g in range(G):
                        i = i0 + g
                        if i % DB == 0:
                            load_chunk(i)
                        xt, off = x_slices.pop(i)
                        ph = phatp.tile([128, PADN], f32, tag="phat")
                        for j in range(NCHUNK):
                            nc.tensor.matmul(
                                ph[:],
                                xt[:, off + j * 128:off + (j + 1) * 128],
                                kp_t[:, j * PADN:(j + 1) * PADN],
                                start=(j == 0),
                                stop=(j == NCHUNK - 1),
                            )
                        ce = cfg.copy_eng[i % len(cfg.copy_eng)]
                        dst = hatw[:, g * S1C:(g + 1) * S1C]
                        if ce == "p":
                            nc.gpsimd.tensor_copy(dst, ph[:, :S1C])
                        elif ce == "v":
                            nc.vector.tensor_copy(dst, ph[:, :S1C])
                        else:
                            nc.scalar.copy(dst, ph[:, :S1C])
                        pump(cfg.pump)
                    active.append(routing_gen(i0, G, gi, hatw))
                    i0 += G
                if cfg.drain_per_rep:
                    drain_all()

            def drain_all():
                while active:
                    pump(1)

            if cfg.loop_reps > 0:
                # hardware loop around an unrolled body: the For_i barrier
                # cost amortizes over cfg.reps unrolled pipeline repeats
                with tc.For_i(0, cfg.loop_reps, 1,
                              hint_engines=(mybir.EngineType.PE,)):
                    for _rep in range(cfg.reps):
                        run_all()
                    drain_all()
            else:
                for _rep in range(cfg.reps):
                    run_all()
                drain_all()

    nc.compile()
    _unify_act_tables(nc)
    return nc


def _unify_act_tables(nc):
    """Replace the greedy per-func LoadActFuncSet placement with a single
    load of a set containing every activation func this kernel uses
    (Copy/Identity/Square/Exp/Ln all live in natural_log_exp_and_others).
    The greedy pass alternates exp_and_others <-> natural_log per squash,
    costing ~1.3us per reload on the ACT engine."""
    from concourse.hw_specs import get_activation_tables

    AF = mybir.ActivationFunctionType
    needed = {AF.Copy, AF.Identity, AF.Square, AF.Exp, AF.Ln}
    tables = list(get_activation_tables(nc.m.arch).items())
    combined_id = None
    for idx, (name, funcs) in enumerate(tables):
        if needed <= funcs:
            combined_id = idx
            break
    if combined_id is None:
        return  # no single set covers us; keep the pass's placement

    fn = nc.m.functions[0]
    blocks = list(fn.blocks)
    kept = None
    for blk in blocks:
        insts = list(blk.instructions)
        out = []
        for i in insts:
            if isinstance(i, mybir.InstLoadActFuncSet):
                if kept is None:
                    i.act_func_set_id = combined_id
                    kept = i
                continue  # drop (re-inserted once below)
            out.append(i)
        if len(out) != len(insts):
            blk.instructions[:] = out
    if kept is None:
        return
    # Place the single load in the entry block (before its terminator) so
    # it dominates every activation, including those inside For_i loops.
    entry = blocks[0]
    insts = list(entry.instructions)
    pos = len(insts)
    while pos > 0 and insts[pos - 1].opcode in ("UnconditionalBranch",
                                                "ConditionalBranch"):
        pos -= 1
    entry.instructions[:] = insts[:pos] + [kept] + insts[pos:]


# ---------------- numpy reference (per-core) ----------------

def ref_numpy(x: np.ndarray, kernel: np.ndarray) -> np.ndarray:
    b = x.shape[0]
    hat = (x @ kernel).reshape(b, NCAP, DCAP)
    logits = np.zeros((b, NCAP, 1), dtype=x.dtype)
    out = None
    for _ in range(3):
        ex = np.exp(logits - logits.max(axis=1, keepdims=True))
        c = ex / ex.sum(axis=1, keepdims=True)
        s = (c * hat).sum(axis=1, keepdims=True)
        s2 = np.square(s).sum(axis=-1, keepdims=True)
        out = s2 / (1.0 + s2) / np.sqrt(s2 + EPS) * s
        logits = logits + np.einsum("bnd,bd->bn", hat, out[:, 0, :])[:, :, None]
    return out[:, 0, :]


# ---------------- public entry point ----------------

_CACHE = {}

BEST = Cfg(n_btiles=16, group_sizes="8,8", big_pat="dddddddd", yraw=1)


def prep_in_maps(x: np.ndarray, kern: np.ndarray, cfg: Cfg):
    kpad = make_kpad(np.asarray(kern, dtype=np.float32), cfg.pad_n)
    return [
        {"xin": prep_x(x[i * cfg.bc:(i + 1) * cfg.bc], cfg.n_btiles),
         "kpad": kpad}
        for i in range(cfg.n_cores)
    ]


def kernel(inputs: np.ndarray, kernel: np.ndarray) -> np.ndarray:
    """CapsuleLayer forward: inputs [16384, 2048] f32, kernel [2048, 160] f32
    -> [16384, 16] f32. Runs SPMD across 8 NeuronCores (batch split 8 ways)."""
    from concourse.bass_utils import run_bass_kernel_spmd

    cfg = BEST
    assert inputs.shape == (cfg.bc * cfg.n_cores, D)
    assert kernel.shape == (D, NCOL)
    if "nc" not in _CACHE:
        _CACHE["nc"] = build(cfg)
    nc = _CACHE["nc"]

    x = np.ascontiguousarray(inputs, dtype=np.float32)
    in_maps = prep_in_maps(x, kernel, cfg)
    res = run_bass_kernel_spmd(nc, in_maps, list(range(cfg.n_cores)))
    outs = []
    for i in range(cfg.n_cores):
        y = res.results[i]["yout"]
        if cfg.yraw:
            # [p, t*16+d] -> [t*128+p, d]
            y = np.ascontiguousarray(
                y.reshape(128, cfg.n_btiles, DCAP).transpose(1, 0, 2)
            ).reshape(cfg.bc, DCAP)
        outs.append(y)
    return np.concatenate(outs, axis=0).astype(np.float32)

